# revision 3
# baseline (speedup 1.0000x reference)
"""SSD-style CustomLoss (Huber loc loss + hard-negative-mined CE conf loss)
as a Trainium2 Bass/Tile kernel, data-parallel over the batch axis on 8
NeuronCores.

Per-core device work (8 images each):
  - CE-from-logits (logsumexp - <y, x>) per box, used to rank negatives
  - CE-from-probs (normalize / clip / log) per box
  - Huber loc loss on positive boxes
  - per-image top-k negative selection via on-device threshold bisection
  - masked sums -> 3 scalar partials per core
Host: pad/reshape inputs, gather the per-core scalar partials, all-reduce
total_pos, final division.
"""

import os
import sys
import types

import numpy as np


def _ensure_ntff_hook():
    """bass_utils' axon trace path imports antenv.axon_hooks, which this
    image lacks. Synthesize the tiny get/set registry and install the
    ctypes-based NTFF hook the boot would have registered."""
    try:
        from antenv.axon_hooks import get_axon_ntff_profile_hook  # noqa: F401

        return
    except ImportError:
        pass
    try:
        import antenv
        from trn_agent_boot.trn_boot import _ntff_profile_via_ctypes

        m = types.ModuleType("antenv.axon_hooks")
        _reg = [None]
        m.set_axon_ntff_profile_hook = lambda h: _reg.__setitem__(0, h)
        m.get_axon_ntff_profile_hook = lambda: _reg[0]
        sys.modules["antenv.axon_hooks"] = m
        antenv.axon_hooks = m
        m.set_axon_ntff_profile_hook(
            _ntff_profile_via_ctypes("/opt/axon/libaxon_pjrt.so")
        )
    except Exception:
        pass

import concourse.bass as bass
import concourse.mybir as mybir
from concourse.bass_utils import run_bass_kernel_spmd
from concourse.mybir import ActivationFunctionType as Act
from concourse.mybir import AluOpType as Op
from concourse.tile import TileContext, add_dep_helper

B, N, C = 64, 8732, 21
NCORES = 8
NIMG = B // NCORES  # images per core
F = 69  # tokens per partition (padded): 128 * 69 = 8832 >= 8732
NPAD = 128 * F
NEG_POS_RATIO = 3.0
EPS = 1e-7
BIG_NEG = -1.0e30
T_BISECT = 18
BISECT_BOUND = 256.0  # |mr| is bounded by ~max|lse| + C*max|y*x| << 256 here
F32 = mybir.dt.float32
X = mybir.AxisListType.X
XY = mybir.AxisListType.XY

# Results of the last device run (exec time etc), for the test harness.
LAST_RESULTS = None

# The walrus build in this container rejects instructions carrying more than
# MAX_WAITS semaphore waits ("Too many sync wait commands"). Tile's scheduler
# freely emits 3+ waits per instruction, so split the excess onto NoOps
# inserted just before the offending instruction (same engine => executes
# before it in the engine's program order).
MAX_WAITS = 1       # per compute/DMA instruction
NOP_WAITS = 1       # per inserted NoOp (same 1-wait limit)


def _split_excess_waits(bir_json: bytes) -> bytes:
    import json as _json

    m = _json.loads(bir_json)
    ctr = 0
    for fdef in m["functions"]:
        for blk in fdef["blocks"]:
            insts = blk["instructions"]
            out = []
            for ins in insts:
                si = ins.get("sync_info")
                ow = (si or {}).get("on_wait") or []
                cap = NOP_WAITS if ins.get("opcode") in ("NoOp", "Drain") else MAX_WAITS
                if len(ow) > cap:
                    keep = ow[-cap:]
                    excess = ow[:-cap]
                    si["on_wait"] = keep
                    while excess:
                        chunk, excess = excess[:NOP_WAITS], excess[NOP_WAITS:]
                        ctr += 1
                        out.append(
                            {
                                "debug": ins.get("debug"),
                                "engine": ins["engine"],
                                "ins": [],
                                "name": f"I-wsplit-{ctr}",
                                "opcode": "NoOp",
                                "outs": [],
                                "sync_info": {"on_update": [], "on_wait": chunk},
                            }
                        )
                out.append(ins)
            blk["instructions"] = out
    return _json.dumps(m).encode()


def _patch_wait_splitting(nc):
    orig = nc.to_json_bytes

    def patched():
        return _split_excess_waits(orig())

    nc.to_json_bytes = patched
    return nc


def emit_program(nc, pl, al, pd, ad, msk, out, n_img, f):
    """Emit the per-core program. pl/al: [n_img, 128*f, C]; pd/ad:
    [n_img, 128*f, 4]; msk: [128, f] (1 = real token); out: [1, 4] =
    (sum hub4*pos, sum <y,log p>*sel, total_pos, unused)."""
    fc = f * C
    f4 = f * 4

    from contextlib import ExitStack

    with TileContext(nc) as tc, ExitStack() as stk:
        per = stk.enter_context(tc.tile_pool(name="per", bufs=1))
        ip = stk.enter_context(tc.tile_pool(name="img", bufs=3))
        pp = stk.enter_context(tc.tile_pool(name="ps", bufs=2, space="PSUM"))

        mskt = per.tile([128, f], F32)
        nc.sync.dma_start(mskt[:], msk[:])

        # persistent per-core maps
        mrm = per.tile([128, n_img * f], F32)   # masked ranking values
        cp = per.tile([128, n_img * f], F32)    # <y, log p> per box
        posm = per.tile([128, n_img * f], F32)  # positive mask
        hpp = per.tile([128, n_img * f], F32)   # hub4 * pos
        pc_img = per.tile([128, n_img], F32)    # per-partition pos counts
        ones128 = per.tile([128, 128], F32)
        nc.vector.memset(ones128[:], 1.0)
        nc.gpsimd.memset(mrm[:], BIG_NEG)

        for b in range(n_img):
            xt = ip.tile([128, fc], F32, tag="xt")
            at = ip.tile([128, fc], F32, tag="at")
            pdt = ip.tile([128, f4], F32, tag="pdt")
            adt = ip.tile([128, f4], F32, tag="adt")
            nc.sync.dma_start(xt[:], pl[b].rearrange("(p f) c -> p (f c)", p=128))
            nc.sync.dma_start(at[:], al[b].rearrange("(p f) c -> p (f c)", p=128))
            nc.sync.dma_start(pdt[:], pd[b].rearrange("(p f) c -> p (f c)", p=128))
            nc.sync.dma_start(adt[:], ad[b].rearrange("(p f) c -> p (f c)", p=128))

            x3 = xt[:].rearrange("p (f c) -> p f c", c=C)
            a3 = at[:].rearrange("p (f c) -> p f c", c=C)
            bf = slice(b * f, (b + 1) * f)

            # --- Huber (sum over the 4 coords; /4 folded into host) ---
            # hub4 = sum_4 (0.5*m^2 - m + |d|), m = min(|d|, 1)
            d3v = lambda t: t[:].rearrange("p (f c) -> p f c", c=4)
            dd = ip.tile([128, f4], F32, tag="dd")
            nc.vector.tensor_sub(dd[:], pdt[:], adt[:])
            absd = ip.tile([128, f4], F32, tag="absd")
            nc.scalar.activation(absd[:], dd[:], Act.Abs)
            m = ip.tile([128, f4], F32, tag="m")
            nc.vector.tensor_scalar_min(m[:], absd[:], 1.0)
            t1 = ip.tile([128, f4], F32, tag="t1")
            # t1 = (m - 2) * m
            nc.vector.scalar_tensor_tensor(t1[:], m[:], -2.0, m[:], Op.add, Op.mult)
            q = ip.tile([128, f4], F32, tag="q")
            # q = 0.5*t1 + |d| = 0.5 m^2 - m + |d|
            nc.vector.scalar_tensor_tensor(q[:], t1[:], 0.5, absd[:], Op.mult, Op.add)
            hub = ip.tile([128, f], F32, tag="hub")
            nc.vector.reduce_sum(hub[:], d3v(q), axis=X)
            # positives: any |actual delta| > 0
            absa = ip.tile([128, f4], F32, tag="absa")
            nc.scalar.activation(absa[:], adt[:], Act.Abs)
            pm = ip.tile([128, f], F32, tag="pm")
            nc.vector.tensor_reduce(pm[:], d3v(absa), axis=X, op=Op.max)
            nc.vector.tensor_scalar(posm[:, bf], pm[:], 0.0, None, Op.is_gt)
            nc.vector.tensor_mul(hpp[:, bf], hub[:], posm[:, bf])
            nc.vector.reduce_sum(pc_img[:, b : b + 1], posm[:, bf], axis=X)

            # --- CE from logits: mr = log(sum exp x) - <y, x> ---
            e = ip.tile([128, fc], F32, tag="e")
            nc.scalar.activation(e[:], xt[:], Act.Exp)
            s1 = ip.tile([128, f], F32, tag="s1")
            nc.vector.reduce_sum(s1[:], e[:].rearrange("p (f c) -> p f c", c=C), axis=X)
            axp = ip.tile([128, fc], F32, tag="axp")
            nc.gpsimd.tensor_mul(axp[:], at[:], xt[:])
            ax = ip.tile([128, f], F32, tag="ax")
            nc.vector.reduce_sum(ax[:], axp[:].rearrange("p (f c) -> p f c", c=C), axis=X)
            lse = ip.tile([128, f], F32, tag="lse")
            nc.scalar.activation(lse[:], s1[:], Act.Ln)
            mr = ip.tile([128, f], F32, tag="mr")
            nc.vector.tensor_sub(mr[:], lse[:], ax[:])

            # --- CE from probs: cp = <y, log clip(x / sum x)> ---
            s2 = ip.tile([128, f], F32, tag="s2")
            nc.vector.reduce_sum(s2[:], x3, axis=X)
            r2 = ip.tile([128, f], F32, tag="r2")
            nc.vector.reciprocal(r2[:], s2[:])
            p = ip.tile([128, fc], F32, tag="p")
            r2b = r2[:, :, None].broadcast_to([128, f, C])
            nc.gpsimd.tensor_tensor(
                p[:].rearrange("p (f c) -> p f c", c=C), x3, r2b, op=Op.mult
            )
            nc.vector.tensor_scalar(p[:], p[:], EPS, 1.0 - EPS, Op.max, Op.min)
            lp = ip.tile([128, fc], F32, tag="lp")
            nc.scalar.activation(lp[:], p[:], Act.Ln)
            alpp = ip.tile([128, fc], F32, tag="alpp")
            nc.gpsimd.tensor_mul(alpp[:], at[:], lp[:])
            nc.vector.reduce_sum(
                cp[:, bf], alpp[:].rearrange("p (f c) -> p f c", c=C), axis=X
            )

            # --- ranking mask: valid negatives only ---
            nv = ip.tile([128, f], mybir.dt.int32, tag="nv")
            nc.vector.tensor_sub(nv[:], mskt[:], posm[:, bf])
            nc.vector.copy_predicated(mrm[:, bf], nv[:], mr[:])

        # ---- cross-partition totals ----
        kps = pp.tile([128, n_img], F32)
        nc.tensor.matmul(kps[:], ones128[:], pc_img[:], start=True, stop=True)
        kimg = per.tile([128, n_img], F32)
        nc.vector.tensor_scalar(kimg[:], kps[:], NEG_POS_RATIO, None, Op.mult)

        # ---- bisection for per-image rank-k threshold ----
        # lo-only form: interval [lo, lo + 2*w_t) with w_t = BOUND/2^t a
        # compile-time constant, so no hi state and one predicated update.
        lo_t = per.tile([128, n_img], F32)
        nc.vector.memset(lo_t[:], -BISECT_BOUND)

        mr3 = mrm[:].rearrange("p (b f) -> p b f", b=n_img)
        mid = per.tile([128, n_img], F32)
        cmp_t = per.tile([128, n_img * f], F32)
        cmp3 = cmp_t[:].rearrange("p (b f) -> p b f", b=n_img)
        cnt = per.tile([128, n_img], F32)
        ge = per.tile([128, n_img], mybir.dt.int32)
        w = BISECT_BOUND
        for _t in range(T_BISECT):
            nc.vector.tensor_scalar_add(mid[:], lo_t[:], w)
            w *= 0.5
            for b in range(n_img):
                bf = slice(b * f, (b + 1) * f)
                nc.vector.tensor_scalar(
                    cmp_t[:, bf], mrm[:, bf], mid[:, b : b + 1], 0.0, Op.is_ge,
                    Op.add, accum_out=cnt[:, b : b + 1],
                )
            cps = pp.tile([128, n_img], F32, tag="cps")
            nc.tensor.matmul(cps[:], ones128[:], cnt[:], start=True, stop=True)
            nc.vector.tensor_tensor(ge[:], cps[:], kimg[:], op=Op.is_ge)
            nc.vector.copy_predicated(lo_t[:], ge[:], mid[:])

        # ---- final masked sums ----
        lob = lo_t[:, :, None].broadcast_to([128, n_img, f])
        nc.vector.tensor_tensor(cmp3, mr3, lob, op=Op.is_ge)  # selected negs
        nc.vector.tensor_add(cmp_t[:], cmp_t[:], posm[:])     # | positives
        sc = per.tile([128, n_img * f], F32)
        csum = per.tile([128, 1], F32)
        nc.vector.tensor_mul(sc[:], cp[:], cmp_t[:])
        nc.vector.reduce_sum(csum[:], sc[:], axis=X)
        hsum = per.tile([128, 1], F32)
        nc.vector.reduce_sum(hsum[:], hpp[:], axis=X)
        ptot = per.tile([128, 1], F32)
        nc.vector.reduce_sum(ptot[:], pc_img[:], axis=X)

        pk = per.tile([128, 4], F32)
        nc.vector.memset(pk[:], 0.0)
        nc.vector.tensor_copy(pk[:, 0:1], hsum[:])
        nc.vector.tensor_copy(pk[:, 1:2], csum[:])
        nc.vector.tensor_copy(pk[:, 2:3], ptot[:])
        pkr = pp.tile([128, 4], F32)
        nc.tensor.matmul(pkr[:], ones128[:], pk[:], start=True, stop=True)
        outt = per.tile([1, 4], F32)
        i_cp = nc.vector.tensor_copy(outt[:], pkr[0:1, :])
        i_dma = nc.sync.dma_start(out[:], outt[:])

        # funnel waits so the tail drain needs few sem waits
        n1 = nc.sync.nop()
        add_dep_helper(n1.ins, i_cp.ins, sync=True, reason="funnel-dve")
        n2 = nc.sync.nop()
        add_dep_helper(n2.ins, i_dma.ins, sync=True, reason="funnel-dma")

    return nc


def build_bass(n_img=NIMG, f=F):
    np_tok = 128 * f
    nc = bass.Bass()
    pl = nc.dram_tensor("pl", [n_img, np_tok, C], F32, kind="ExternalInput")
    al = nc.dram_tensor("al", [n_img, np_tok, C], F32, kind="ExternalInput")
    pd = nc.dram_tensor("pd", [n_img, np_tok, 4], F32, kind="ExternalInput")
    ad = nc.dram_tensor("ad", [n_img, np_tok, 4], F32, kind="ExternalInput")
    msk = nc.dram_tensor("msk", [128, f], F32, kind="ExternalInput")
    out = nc.dram_tensor("out", [1, 4], F32, kind="ExternalOutput")
    emit_program(nc, pl, al, pd, ad, msk, out, n_img, f)
    return _patch_wait_splitting(nc)


def _pad_tokens(x, npad, fill):
    """[B, N, D] -> [B, npad, D] padded with `fill` along tokens."""
    b, n, d = x.shape
    if n == npad:
        return np.ascontiguousarray(x, dtype=np.float32)
    out = np.full((b, npad, d), fill, dtype=np.float32)
    out[:, :n, :] = x
    return out


def kernel(actual_bbox_deltas, actual_labels, pred_bbox_deltas, pred_labels):
    global LAST_RESULTS
    ab = np.asarray(actual_bbox_deltas, dtype=np.float32)
    al_ = np.asarray(actual_labels, dtype=np.float32)
    pb = np.asarray(pred_bbox_deltas, dtype=np.float32)
    pl_ = np.asarray(pred_labels, dtype=np.float32)
    assert pl_.shape == (B, N, C), pl_.shape

    # Pad tokens to 128*F. Padded pred_labels rows are all-ones (safe for
    # exp/log); padded labels/deltas are zero, and the msk input excludes
    # padded tokens from negative mining.
    plp = _pad_tokens(pl_, NPAD, 1.0)
    alp = _pad_tokens(al_, NPAD, 0.0)
    pbp = _pad_tokens(pb, NPAD, 0.0)
    abp = _pad_tokens(ab, NPAD, 0.0)

    tok = np.arange(NPAD).reshape(128, F)
    msk = (tok < N).astype(np.float32)

    nc = build_bass()
    in_maps = []
    for c in range(NCORES):
        sl = slice(c * NIMG, (c + 1) * NIMG)
        in_maps.append(
            {
                "pl": np.ascontiguousarray(plp[sl]),
                "al": np.ascontiguousarray(alp[sl]),
                "pd": np.ascontiguousarray(pbp[sl]),
                "ad": np.ascontiguousarray(abp[sl]),
                "msk": msk,
            }
        )

    trace = bool(int(os.environ.get("KERNEL_TRACE", "0")))
    if trace:
        _ensure_ntff_hook()
    res = run_bass_kernel_spmd(
        nc, in_maps, core_ids=list(range(NCORES)), trace=trace
    )
    LAST_RESULTS = res

    hub_sum = 0.0
    cesel_sum = 0.0
    pos_total = 0.0
    for r in res.results:
        o = r["out"].reshape(-1)
        hub_sum += float(o[0])
        cesel_sum += float(o[1])
        pos_total += float(o[2])

    total_pos = max(pos_total, 1.0)
    loc_loss = np.float32(0.25 * hub_sum / total_pos)
    conf_loss = np.float32(-cesel_sum / total_pos)
    return loc_loss, conf_loss



# revision 9
# speedup vs baseline: 2.0712x; 2.0712x over previous
"""SSD-style CustomLoss (Huber loc loss + hard-negative-mined CE conf loss)
as a Trainium2 Bass/Tile kernel, data-parallel over the batch axis on 8
NeuronCores.

v4 design (vs. baseline):
  - bf16 inputs (halves HBM traffic; label one-hots and nonzero-ness of
    deltas are preserved exactly by the cast).
  - CE-from-probs collapses via the one-hot identity to
    cp = ln(clip(<y,x>/sum_c x, eps, 1-eps)) -- no per-class normalize/
    clip/log tensors.
  - Partition layout p = (image, 16 sub-rows); tokens stream in 4 chunks
    of 2208 tokens x 8 images. All heavy elementwise work runs on
    [128, 2898]-shaped tiles (big instructions, few bubbles).
  - Per-token sums over C=21 via bf16 tensor_tensor trees (2x DVE mode)
    instead of 1x tensor_reduce.
  - Huber in exact relu^2 form: hub = 0.5 d^2 - 0.5 relu(d-1)^2
    - 0.5 min(d+1, 0)^2, with Square+accum on the scalar engine.
  - Hard-negative threshold via 12-round bisection; per-image counts with
    ONE tensor_scalar+accum per round thanks to the image-major partition
    layout, cross-partition block-sum via a blockdiag ones matmul.
"""

import os
import sys
import types

import numpy as np

import concourse.bass as bass
import concourse.mybir as mybir
from concourse.bass_utils import run_bass_kernel_spmd
from concourse.mybir import ActivationFunctionType as Act
from concourse.mybir import AluOpType as Op
from concourse.tile import TileContext, add_dep_helper

B, N, C = 64, 8732, 21
NCORES = 8
NIMG = B // NCORES          # images per core
SUBS = 16                   # sub-rows per image -> 8*16 = 128 partitions
TPS = 552                   # tokens per sub-row: 16*552 = 8832 >= 8732
NPAD = SUBS * TPS           # padded tokens per image
NCHUNK = 4
TPC = TPS // NCHUNK         # tokens per partition per chunk = 138
FC = TPC * C                # 2898 label elems per partition per chunk
FD = TPC * 8                # 1104 delta elems per partition per chunk
EPS = 1e-7
BIG = 1.0e30
PLPAD = -20.0               # pl value for padded tokens
T_BISECT = 12
BISECT_LO = 1.0
BISECT_W0 = 8.0             # interval [lo, lo+2w); resolution 16/2^12
F32 = mybir.dt.float32
BF16 = mybir.dt.bfloat16
NPBF16 = np.dtype(mybir.dt.np(mybir.dt.bfloat16))
X = mybir.AxisListType.X

LAST_RESULTS = None

# The walrus build in this container rejects instructions carrying more than
# MAX_WAITS semaphore waits. Tile's scheduler freely emits more, so split the
# excess onto NoOps inserted just before the offending instruction.
MAX_WAITS = 1
NOP_WAITS = 1


def _ensure_ntff_hook():
    """bass_utils' axon trace path imports antenv.axon_hooks, which this
    image lacks. Synthesize the tiny get/set registry and install the
    ctypes-based NTFF hook the boot would have registered."""
    try:
        from antenv.axon_hooks import get_axon_ntff_profile_hook  # noqa: F401

        return
    except ImportError:
        pass
    try:
        import antenv
        from trn_agent_boot.trn_boot import _ntff_profile_via_ctypes

        m = types.ModuleType("antenv.axon_hooks")
        _reg = [None]
        m.set_axon_ntff_profile_hook = lambda h: _reg.__setitem__(0, h)
        m.get_axon_ntff_profile_hook = lambda: _reg[0]
        sys.modules["antenv.axon_hooks"] = m
        antenv.axon_hooks = m
        m.set_axon_ntff_profile_hook(
            _ntff_profile_via_ctypes("/opt/axon/libaxon_pjrt.so")
        )
    except Exception:
        pass


def _split_excess_waits(bir_json: bytes) -> bytes:
    import json as _json

    m = _json.loads(bir_json)
    ctr = 0
    for fdef in m["functions"]:
        for blk in fdef["blocks"]:
            insts = blk["instructions"]
            out = []
            for ins in insts:
                si = ins.get("sync_info")
                ow = (si or {}).get("on_wait") or []
                cap = NOP_WAITS if ins.get("opcode") in ("NoOp", "Drain") else MAX_WAITS
                if len(ow) > cap:
                    keep = ow[-cap:]
                    excess = ow[:-cap]
                    si["on_wait"] = keep
                    while excess:
                        chunk, excess = excess[:NOP_WAITS], excess[NOP_WAITS:]
                        ctr += 1
                        out.append(
                            {
                                "debug": ins.get("debug"),
                                "engine": ins["engine"],
                                "ins": [],
                                "name": f"I-wsplit-{ctr}",
                                "opcode": "NoOp",
                                "outs": [],
                                "sync_info": {"on_update": [], "on_wait": chunk},
                            }
                        )
                out.append(ins)
            blk["instructions"] = out
    return _json.dumps(m).encode()


def _patch_wait_splitting(nc):
    orig = nc.to_json_bytes

    def patched():
        return _split_excess_waits(orig())

    nc.to_json_bytes = patched
    return nc


def _col(ap3, j):
    """[128, T, W] view -> [128, T] view of column j (stride W)."""
    return ap3[:, :, j : j + 1].rearrange("p t o -> p (t o)")


def _tree21(nc, pool, src3, out_f32, tag):
    """Per-token sum over C=21 of src3 ([128, TPC, 21] view, bf16) into
    out_f32 ([128, TPC] f32) via a tensor_tensor add tree (2x DVE mode on
    the wide levels)."""
    t10 = pool.tile([128, TPC, 10], BF16, tag=f"{tag}_t10")
    nc.vector.tensor_tensor(
        t10[:], src3[:, :, 0:10], src3[:, :, 10:20], op=Op.add
    )
    t5 = pool.tile([128, TPC, 5], BF16, tag=f"{tag}_t5")
    nc.vector.tensor_tensor(t5[:], t10[:, :, 0:5], t10[:, :, 5:10], op=Op.add)
    t2 = pool.tile([128, TPC, 2], BF16, tag=f"{tag}_t2")
    nc.vector.tensor_tensor(t2[:], t5[:, :, 0:2], t5[:, :, 2:4], op=Op.add)
    sA = pool.tile([128, TPC], F32, tag=f"{tag}_sA")
    nc.vector.tensor_tensor(sA[:], _col(t2, 0), _col(t2, 1), op=Op.add)
    sB = pool.tile([128, TPC], F32, tag=f"{tag}_sB")
    nc.vector.tensor_tensor(sB[:], _col(t5, 4), _col(src3, 20), op=Op.add)
    nc.vector.tensor_tensor(out_f32[:], sA[:], sB[:], op=Op.add)


def emit_program(nc, xl, al, dl, w16, ones, out):
    from contextlib import ExitStack

    with TileContext(nc) as tc, ExitStack() as stk:
        per = stk.enter_context(tc.tile_pool(name="per", bufs=1))
        cp_pool = stk.enter_context(tc.tile_pool(name="chunk", bufs=2))
        pp = stk.enter_context(tc.tile_pool(name="ps", bufs=2, space="PSUM"))

        w16t = per.tile([128, 128], F32)
        nc.sync.dma_start(w16t[:], w16[:])
        onest = per.tile([128, 128], F32)
        nc.sync.dma_start(onest[:], ones[:])

        # persistent per-core state
        mrm = per.tile([128, TPS], F32)     # masked ranking values (mr)
        cpt = per.tile([128, TPS], F32)     # ln(clip(x_k / s2)) per token
        posm = per.tile([128, TPS], BF16)   # positive mask
        pc = per.tile([128, NCHUNK], F32)   # per-partition pos counts by chunk
        d2s = per.tile([128, NCHUNK], F32)  # sum d^2*pos by chunk
        r1s = per.tile([128, NCHUNK], F32)  # sum relu(d-1)^2*pos
        r2s = per.tile([128, NCHUNK], F32)  # sum min(d+1,0)^2*pos

        for k in range(NCHUNK):
            ksl = slice(k * TPC, (k + 1) * TPC)
            xt = cp_pool.tile([128, FC], BF16, tag="xt")
            at = cp_pool.tile([128, FC], BF16, tag="at")
            dt = cp_pool.tile([128, FD], BF16, tag="dt")
            nc.sync.dma_start(xt[:], xl[k])
            nc.sync.dma_start(at[:], al[k])
            nc.sync.dma_start(dt[:], dl[k])

            x3 = xt[:].rearrange("p (t c) -> p t c", c=C)
            a3 = at[:].rearrange("p (t c) -> p t c", c=C)
            d3 = dt[:].rearrange("p (t c) -> p t c", c=8)

            # --- CE stats: s1 = sum exp x, ax = <y,x>, s2 = sum x ---
            et = cp_pool.tile([128, FC], BF16, tag="et")
            nc.scalar.activation(et[:], xt[:], Act.Exp)
            axt = cp_pool.tile([128, FC], BF16, tag="axt")
            nc.vector.tensor_mul(axt[:], at[:], xt[:])
            s1 = cp_pool.tile([128, TPC], F32, tag="s1")
            _tree21(nc, cp_pool, et[:].rearrange("p (t c) -> p t c", c=C), s1, "e")
            ax = cp_pool.tile([128, TPC], F32, tag="ax")
            _tree21(nc, cp_pool, axt[:].rearrange("p (t c) -> p t c", c=C), ax, "a")
            s2 = cp_pool.tile([128, TPC], F32, tag="s2")
            _tree21(nc, cp_pool, x3, s2, "x")

            lse = cp_pool.tile([128, TPC], F32, tag="lse")
            nc.scalar.activation(lse[:], s1[:], Act.Ln)
            mr = cp_pool.tile([128, TPC], F32, tag="mr")
            nc.vector.tensor_sub(mr[:], lse[:], ax[:])

            # --- cp = ln(clip(ax / s2)) ---
            r2v = cp_pool.tile([128, TPC], F32, tag="r2v")
            nc.vector.reciprocal(r2v[:], s2[:])
            ratio = cp_pool.tile([128, TPC], F32, tag="ratio")
            nc.vector.tensor_mul(ratio[:], ax[:], r2v[:])
            rc = cp_pool.tile([128, TPC], F32, tag="rc")
            nc.vector.tensor_scalar(rc[:], ratio[:], EPS, 1.0 - EPS, Op.max, Op.min)
            nc.scalar.activation(cpt[:, ksl], rc[:], Act.Ln)

            # --- positives: any |actual delta| > 0 (via sum of squares) ---
            sq = cp_pool.tile([128, TPC, 4], BF16, tag="sq")
            nc.scalar.activation(
                sq[:], d3[:, :, 4:8], Act.Square
            )
            p1 = cp_pool.tile([128, TPC, 2], BF16, tag="p1")
            nc.vector.tensor_tensor(p1[:], sq[:, :, 0:2], sq[:, :, 2:4], op=Op.add)
            s4 = cp_pool.tile([128, TPC], F32, tag="s4")
            nc.vector.tensor_tensor(s4[:], _col(p1, 0), _col(p1, 1), op=Op.add)
            nc.vector.tensor_scalar(
                posm[:, ksl], s4[:], 0.0, 0.0, Op.is_gt, Op.add,
                accum_out=pc[:, k : k + 1],
            )

            # --- masked ranking: mrm = mr - BIG*pos (pads rank lowest) ---
            nc.vector.scalar_tensor_tensor(
                mrm[:, ksl], posm[:, ksl], -BIG, mr[:], Op.mult, Op.add
            )

            # --- Huber: hub = 0.5 d^2 - 0.5 relu(d-1)^2 - 0.5 min(d+1,0)^2,
            #     with d pre-masked by pos so negatives contribute 0 ---
            dd = cp_pool.tile([128, TPC, 4], BF16, tag="dd")
            nc.gpsimd.tensor_sub(dd[:], d3[:, :, 0:4], d3[:, :, 4:8])
            dpos = cp_pool.tile([128, TPC, 4], BF16, tag="dpos")
            pos_b = posm[:, ksl][:, :, None].broadcast_to([128, TPC, 4])
            nc.gpsimd.tensor_tensor(dpos[:], dd[:], pos_b, op=Op.mult)

            dsq = cp_pool.tile([128, TPC, 4], BF16, tag="dsq")
            nc.scalar.activation(
                dsq[:], dpos[:], Act.Square, accum_out=d2s[:, k : k + 1]
            )
            re1 = cp_pool.tile([128, TPC, 4], BF16, tag="re1")
            nc.vector.tensor_scalar(re1[:], dpos[:], 1.0, 0.0, Op.subtract, Op.max)
            rsq = cp_pool.tile([128, TPC, 4], BF16, tag="rsq")
            nc.scalar.activation(
                rsq[:], re1[:], Act.Square, accum_out=r1s[:, k : k + 1]
            )
            re2 = cp_pool.tile([128, TPC, 4], BF16, tag="re2")
            nc.vector.tensor_scalar(re2[:], dpos[:], 1.0, 0.0, Op.add, Op.min)
            msq = cp_pool.tile([128, TPC, 4], BF16, tag="msq")
            nc.scalar.activation(
                msq[:], re2[:], Act.Square, accum_out=r2s[:, k : k + 1]
            )

        # ---- per-image positive counts, broadcast within image blocks ----
        pcv = per.tile([128, 1], F32)
        nc.vector.reduce_sum(pcv[:], pc[:], axis=X)
        pcb = pp.tile([128, 1], F32)
        nc.tensor.matmul(pcb[:], w16t[:], pcv[:], start=True, stop=True)
        kimg = per.tile([128, 1], F32)
        nc.vector.tensor_scalar(kimg[:], pcb[:], 3.0, None, Op.mult)

        # ---- bisection for per-image rank-k threshold on mrm ----
        lo = per.tile([128, 1], F32)
        nc.vector.memset(lo[:], BISECT_LO)
        mid = per.tile([128, 1], F32)
        cmpd = per.tile([128, TPS], BF16)
        cnt = per.tile([128, 1], F32)
        ge = per.tile([128, 1], mybir.dt.int32)
        w = BISECT_W0
        for _t in range(T_BISECT):
            nc.vector.tensor_scalar_add(mid[:], lo[:], w)
            w *= 0.5
            nc.vector.tensor_scalar(
                cmpd[:], mrm[:], mid[:], 0.0, Op.is_ge, Op.add,
                accum_out=cnt[:],
            )
            cps = pp.tile([128, 1], F32, tag="cps")
            nc.tensor.matmul(cps[:], w16t[:], cnt[:], start=True, stop=True)
            nc.vector.tensor_tensor(ge[:], cps[:], kimg[:], op=Op.is_ge)
            nc.vector.copy_predicated(lo[:], ge[:], mid[:])

        # ---- final masked sums ----
        selv = per.tile([128, TPS], BF16)
        nc.vector.tensor_scalar(selv[:], mrm[:], lo[:], None, Op.is_ge)
        sel2 = per.tile([128, TPS], BF16)
        nc.vector.tensor_tensor(sel2[:], selv[:], posm[:], op=Op.add)
        scr = per.tile([128, TPS], F32)
        nc.vector.tensor_mul(scr[:], cpt[:], sel2[:])
        scr2 = per.tile([128, TPS], F32)
        csum = per.tile([128, 1], F32)
        nc.vector.tensor_scalar(
            scr2[:], scr[:], 0.0, 0.0, Op.add, Op.add, accum_out=csum[:]
        )

        # hub partial = 0.5*(sum d2 - sum r1 - sum r2) per partition
        htmp = per.tile([128, NCHUNK], F32)
        nc.vector.tensor_sub(htmp[:], d2s[:], r1s[:])
        nc.vector.tensor_sub(htmp[:], htmp[:], r2s[:])
        hsum = per.tile([128, 1], F32)
        nc.vector.reduce_sum(hsum[:], htmp[:], axis=X)

        pk = per.tile([128, 4], F32)
        nc.vector.memset(pk[:], 0.0)
        nc.vector.tensor_copy(pk[:, 0:1], hsum[:])
        nc.vector.tensor_copy(pk[:, 1:2], csum[:])
        nc.vector.tensor_copy(pk[:, 2:3], pcv[:])
        pkr = pp.tile([128, 4], F32)
        nc.tensor.matmul(pkr[:], onest[:], pk[:], start=True, stop=True)
        outt = per.tile([1, 4], F32)
        i_cp = nc.vector.tensor_copy(outt[:], pkr[0:1, :])
        i_dma = nc.sync.dma_start(out[:], outt[:])

        n1 = nc.sync.nop()
        add_dep_helper(n1.ins, i_cp.ins, sync=True, reason="funnel-dve")
        n2 = nc.sync.nop()
        add_dep_helper(n2.ins, i_dma.ins, sync=True, reason="funnel-dma")

    return nc


def build_bass():
    nc = bass.Bass()
    xl = nc.dram_tensor("xl", [NCHUNK, 128, FC], BF16, kind="ExternalInput")
    al = nc.dram_tensor("al", [NCHUNK, 128, FC], BF16, kind="ExternalInput")
    dl = nc.dram_tensor("dl", [NCHUNK, 128, FD], BF16, kind="ExternalInput")
    w16 = nc.dram_tensor("w16", [128, 128], F32, kind="ExternalInput")
    ones = nc.dram_tensor("ones", [128, 128], F32, kind="ExternalInput")
    out = nc.dram_tensor("out", [1, 4], F32, kind="ExternalOutput")
    emit_program(nc, xl, al, dl, w16, ones, out)
    return _patch_wait_splitting(nc)


def _to_chunks(x, fill):
    """[NIMG, N, D] f32 -> [NCHUNK, 128, TPC*D] bf16 in the
    p=(image,sub) / token-chunk layout."""
    nimg, n, dd = x.shape
    buf = np.full((nimg, NPAD, dd), fill, dtype=np.float32)
    buf[:, :n, :] = x
    # token T = s*TPS + k*TPC + pos
    buf = buf.reshape(nimg, SUBS, NCHUNK, TPC, dd)
    buf = buf.transpose(2, 0, 1, 3, 4)  # (k, i, s, pos, d)
    buf = buf.reshape(NCHUNK, 128, TPC * dd)
    return np.ascontiguousarray(buf.astype(NPBF16))


def kernel(actual_bbox_deltas, actual_labels, pred_bbox_deltas, pred_labels):
    global LAST_RESULTS
    ab = np.asarray(actual_bbox_deltas, dtype=np.float32)
    al_ = np.asarray(actual_labels, dtype=np.float32)
    pb = np.asarray(pred_bbox_deltas, dtype=np.float32)
    pl_ = np.asarray(pred_labels, dtype=np.float32)
    assert pl_.shape == (B, N, C), pl_.shape

    # deltas interleaved per token: (pd0..3, ad0..3)
    pdad = np.concatenate([pb, ab], axis=2)  # [B, N, 8]

    blk = np.arange(128) // SUBS
    w16 = (blk[:, None] == blk[None, :]).astype(np.float32)
    ones = np.ones((128, 128), np.float32)

    nc = build_bass()
    in_maps = []
    for c in range(NCORES):
        sl = slice(c * NIMG, (c + 1) * NIMG)
        in_maps.append(
            {
                "xl": _to_chunks(pl_[sl], PLPAD),
                "al": _to_chunks(al_[sl], 0.0),
                "dl": _to_chunks(pdad[sl], 0.0),
                "w16": w16,
                "ones": ones,
            }
        )

    trace = bool(int(os.environ.get("KERNEL_TRACE", "0")))
    if trace:
        _ensure_ntff_hook()
    res = run_bass_kernel_spmd(
        nc, in_maps, core_ids=list(range(NCORES)), trace=trace
    )
    LAST_RESULTS = res

    hub_sum = 0.0
    cesel_sum = 0.0
    pos_total = 0.0
    for r in res.results:
        o = r["out"].reshape(-1)
        hub_sum += float(o[0])
        cesel_sum += float(o[1])
        pos_total += float(o[2])

    total_pos = max(pos_total, 1.0)
    loc_loss = np.float32(0.25 * 0.5 * hub_sum / total_pos)
    conf_loss = np.float32(-cesel_sum / total_pos)
    return loc_loss, conf_loss


# revision 15
# speedup vs baseline: 2.1023x; 1.0150x over previous
"""SSD-style CustomLoss (Huber loc loss + hard-negative-mined CE conf loss)
as a Trainium2 Bass/Tile kernel, data-parallel over the batch axis on 8
NeuronCores.

v4 design (vs. baseline):
  - bf16 inputs (halves HBM traffic; label one-hots and nonzero-ness of
    deltas are preserved exactly by the cast).
  - CE-from-probs collapses via the one-hot identity to
    cp = ln(clip(<y,x>/sum_c x, eps, 1-eps)) -- no per-class normalize/
    clip/log tensors.
  - Partition layout p = (image, 16 sub-rows); tokens stream in 4 chunks
    of 2208 tokens x 8 images. All heavy elementwise work runs on
    [128, 2898]-shaped tiles (big instructions, few bubbles).
  - Per-token sums over C=21 via bf16 tensor_tensor trees (2x DVE mode)
    instead of 1x tensor_reduce.
  - Huber in exact relu^2 form: hub = 0.5 d^2 - 0.5 relu(d-1)^2
    - 0.5 min(d+1, 0)^2, with Square+accum on the scalar engine.
  - Hard-negative threshold via 12-round bisection; per-image counts with
    ONE tensor_scalar+accum per round thanks to the image-major partition
    layout, cross-partition block-sum via a blockdiag ones matmul.
"""

import os
import sys
import types

import numpy as np

import concourse.bass as bass
import concourse.mybir as mybir
from concourse.bass_utils import run_bass_kernel_spmd
from concourse.mybir import ActivationFunctionType as Act
from concourse.mybir import AluOpType as Op
from concourse.tile import TileContext, add_dep_helper

B, N, C = 64, 8732, 21
NCORES = 8
NIMG = B // NCORES          # images per core
SUBS = 16                   # sub-rows per image -> 8*16 = 128 partitions
TPS = 552                   # tokens per sub-row: 16*552 = 8832 >= 8732
NPAD = SUBS * TPS           # padded tokens per image
NCHUNK = 4
TPC = TPS // NCHUNK         # tokens per partition per chunk = 138
FC = TPC * C                # 2898 label elems per partition per chunk
FD = TPC * 8                # 1104 delta elems per partition per chunk
EPS = 1e-7
BIG = 1.0e30
PLPAD = -20.0               # pl value for padded tokens
T_BISECT = 11
BISECT_LO = 1.0
BISECT_W0 = 8.0             # interval [lo, lo+2w); resolution 16/2^11
F32 = mybir.dt.float32
BF16 = mybir.dt.bfloat16
NPBF16 = np.dtype(mybir.dt.np(mybir.dt.bfloat16))
X = mybir.AxisListType.X

LAST_RESULTS = None

# The walrus build in this container rejects instructions carrying more than
# MAX_WAITS semaphore waits. Tile's scheduler freely emits more, so split the
# excess onto NoOps inserted just before the offending instruction.
MAX_WAITS = 1
NOP_WAITS = 1


def _ensure_ntff_hook():
    """bass_utils' axon trace path imports antenv.axon_hooks, which this
    image lacks. Synthesize the tiny get/set registry and install the
    ctypes-based NTFF hook the boot would have registered."""
    try:
        from antenv.axon_hooks import get_axon_ntff_profile_hook  # noqa: F401

        return
    except ImportError:
        pass
    try:
        import antenv
        from trn_agent_boot.trn_boot import _ntff_profile_via_ctypes

        m = types.ModuleType("antenv.axon_hooks")
        _reg = [None]
        m.set_axon_ntff_profile_hook = lambda h: _reg.__setitem__(0, h)
        m.get_axon_ntff_profile_hook = lambda: _reg[0]
        sys.modules["antenv.axon_hooks"] = m
        antenv.axon_hooks = m
        m.set_axon_ntff_profile_hook(
            _ntff_profile_via_ctypes("/opt/axon/libaxon_pjrt.so")
        )
    except Exception:
        pass


def _split_excess_waits(bir_json: bytes) -> bytes:
    import json as _json

    m = _json.loads(bir_json)
    ctr = 0
    for fdef in m["functions"]:
        for blk in fdef["blocks"]:
            insts = blk["instructions"]
            out = []
            for ins in insts:
                si = ins.get("sync_info")
                ow = (si or {}).get("on_wait") or []
                cap = NOP_WAITS if ins.get("opcode") in ("NoOp", "Drain") else MAX_WAITS
                if len(ow) > cap:
                    keep = ow[-cap:]
                    excess = ow[:-cap]
                    si["on_wait"] = keep
                    while excess:
                        chunk, excess = excess[:NOP_WAITS], excess[NOP_WAITS:]
                        ctr += 1
                        out.append(
                            {
                                "debug": ins.get("debug"),
                                "engine": ins["engine"],
                                "ins": [],
                                "name": f"I-wsplit-{ctr}",
                                "opcode": "NoOp",
                                "outs": [],
                                "sync_info": {"on_update": [], "on_wait": chunk},
                            }
                        )
                out.append(ins)
            blk["instructions"] = out
    return _json.dumps(m).encode()


def _patch_wait_splitting(nc):
    orig = nc.to_json_bytes

    def patched():
        return _split_excess_waits(orig())

    nc.to_json_bytes = patched
    return nc


def _col(ap3, j):
    """[128, T, W] view -> [128, T] view of column j (stride W)."""
    return ap3[:, :, j : j + 1].rearrange("p t o -> p (t o)")


def _tree21(nc, pool, src3, out_f32, tag):
    """Per-token sum over C=21 of src3 ([128, TPC, 21] view, bf16) into
    out_f32 ([128, TPC] f32) via a tensor_tensor add tree (2x DVE mode on
    the wide levels)."""
    t10 = pool.tile([128, TPC, 10], BF16, tag=f"{tag}_t10")
    nc.vector.tensor_tensor(
        t10[:], src3[:, :, 0:10], src3[:, :, 10:20], op=Op.add
    )
    t5 = pool.tile([128, TPC, 5], BF16, tag=f"{tag}_t5")
    nc.vector.tensor_tensor(t5[:], t10[:, :, 0:5], t10[:, :, 5:10], op=Op.add)
    t2 = pool.tile([128, TPC, 2], BF16, tag=f"{tag}_t2")
    nc.vector.tensor_tensor(t2[:], t5[:, :, 0:2], t5[:, :, 2:4], op=Op.add)
    sA = pool.tile([128, TPC], F32, tag=f"{tag}_sA")
    nc.vector.tensor_tensor(sA[:], _col(t2, 0), _col(t2, 1), op=Op.add)
    sB = pool.tile([128, TPC], F32, tag=f"{tag}_sB")
    nc.vector.tensor_tensor(sB[:], _col(t5, 4), _col(src3, 20), op=Op.add)
    nc.vector.tensor_tensor(out_f32, sA[:], sB[:], op=Op.add)


def emit_program(nc, xl, al, dl, w16, ones, out):
    from contextlib import ExitStack

    with TileContext(nc) as tc, ExitStack() as stk:
        per = stk.enter_context(tc.tile_pool(name="per", bufs=1))
        cp_pool = stk.enter_context(tc.tile_pool(name="chunk", bufs=2))
        pp = stk.enter_context(tc.tile_pool(name="ps", bufs=2, space="PSUM"))

        # persistent per-core state
        mrm = per.tile([128, TPS], F32)     # masked ranking values (mr)
        cpt = per.tile([128, TPS], F32)     # ln(clip(x_k / s2)) per token
        posm = per.tile([128, TPS], BF16)   # positive mask
        s1a = per.tile([128, TPS], F32)     # sum exp x per token
        axa = per.tile([128, TPS], F32)     # <y, x> per token
        s2a = per.tile([128, TPS], F32)     # sum x per token
        pc = per.tile([128, NCHUNK], F32)   # per-partition pos counts by chunk
        d2s = per.tile([128, NCHUNK], F32)  # sum d^2*pos by chunk
        r1s = per.tile([128, NCHUNK], F32)  # sum relu(d-1)^2*pos
        r2s = per.tile([128, NCHUNK], F32)  # sum min(d+1,0)^2*pos
        w16t = per.tile([128, 128], F32)
        onest = per.tile([128, 128], F32)

        for k in range(NCHUNK):
            ksl = slice(k * TPC, (k + 1) * TPC)
            xt = cp_pool.tile([128, FC], BF16, tag="xt")
            at = cp_pool.tile([128, FC], BF16, tag="at")
            dt = cp_pool.tile([128, FD], BF16, tag="dt")
            nc.sync.dma_start(xt[:], xl[k])
            nc.sync.dma_start(at[:], al[k])
            nc.sync.dma_start(dt[:], dl[k])

            x3 = xt[:].rearrange("p (t c) -> p t c", c=C)
            a3 = at[:].rearrange("p (t c) -> p t c", c=C)
            d3 = dt[:].rearrange("p (t c) -> p t c", c=8)

            # --- CE stats: s1 = sum exp x, ax = <y,x>, s2 = sum x ---
            et = cp_pool.tile([128, FC], BF16, tag="et")
            nc.scalar.activation(et[:], xt[:], Act.Exp)
            axt = cp_pool.tile([128, FC], BF16, tag="axt")
            nc.vector.tensor_mul(axt[:], at[:], xt[:])
            s1 = s1a[:, ksl]
            _tree21(nc, cp_pool, et[:].rearrange("p (t c) -> p t c", c=C), s1, "e")
            ax = axa[:, ksl]
            _tree21(nc, cp_pool, axt[:].rearrange("p (t c) -> p t c", c=C), ax, "a")
            s2 = s2a[:, ksl]
            _tree21(nc, cp_pool, x3, s2, "x")

            # --- positives: any |actual delta| > 0 (via sum of squares) ---
            sq = cp_pool.tile([128, TPC, 4], BF16, tag="sq")
            nc.scalar.activation(
                sq[:], d3[:, :, 4:8], Act.Square
            )
            p1 = cp_pool.tile([128, TPC, 2], BF16, tag="p1")
            nc.vector.tensor_tensor(p1[:], sq[:, :, 0:2], sq[:, :, 2:4], op=Op.add)
            s4 = cp_pool.tile([128, TPC], F32, tag="s4")
            nc.vector.tensor_tensor(s4[:], _col(p1, 0), _col(p1, 1), op=Op.add)
            nc.vector.tensor_scalar(
                posm[:, ksl], s4[:], 0.0, 0.0, Op.is_gt, Op.add,
                accum_out=pc[:, k : k + 1],
            )

            # --- Huber: hub = 0.5 d^2 - 0.5 relu(d-1)^2 - 0.5 min(d+1,0)^2,
            #     with d pre-masked by pos so negatives contribute 0 ---
            dd = cp_pool.tile([128, TPC, 4], BF16, tag="dd")
            nc.gpsimd.tensor_sub(dd[:], d3[:, :, 0:4], d3[:, :, 4:8])
            dpos = cp_pool.tile([128, TPC, 4], BF16, tag="dpos")
            pos_b = posm[:, ksl][:, :, None].broadcast_to([128, TPC, 4])
            nc.gpsimd.tensor_tensor(dpos[:], dd[:], pos_b, op=Op.mult)

            dsq = cp_pool.tile([128, TPC, 4], BF16, tag="dsq")
            nc.scalar.activation(
                dsq[:], dpos[:], Act.Square, accum_out=d2s[:, k : k + 1]
            )
            re1 = cp_pool.tile([128, TPC, 4], BF16, tag="re1")
            nc.vector.tensor_scalar(re1[:], dpos[:], 1.0, 0.0, Op.subtract, Op.max)
            rsq = cp_pool.tile([128, TPC, 4], BF16, tag="rsq")
            nc.scalar.activation(
                rsq[:], re1[:], Act.Square, accum_out=r1s[:, k : k + 1]
            )
            re2 = cp_pool.tile([128, TPC, 4], BF16, tag="re2")
            nc.vector.tensor_scalar(re2[:], dpos[:], 1.0, 0.0, Op.add, Op.min)
            msq = cp_pool.tile([128, TPC, 4], BF16, tag="msq")
            nc.scalar.activation(
                msq[:], re2[:], Act.Square, accum_out=r2s[:, k : k + 1]
            )

        # ---- weights for cross-partition sums (needed from here on) ----
        nc.sync.dma_start(w16t[:], w16[:])
        nc.sync.dma_start(onest[:], ones[:])

        # ---- batched ranking values: mr = ln(s1) - ax, minus BIG at pos ----
        lse = per.tile([128, TPS], F32)
        nc.scalar.activation(lse[:], s1a[:], Act.Ln)
        mrv = per.tile([128, TPS], F32)
        nc.vector.tensor_sub(mrv[:], lse[:], axa[:])
        negm = per.tile([128, TPS], BF16)
        nc.vector.tensor_scalar(negm[:], posm[:], -BIG, None, Op.mult)
        nc.vector.tensor_add(mrm[:], mrv[:], negm[:])

        # ---- per-image positive counts, broadcast within image blocks ----
        pcv = per.tile([128, 1], F32)
        nc.vector.reduce_sum(pcv[:], pc[:], axis=X)
        pcb = pp.tile([128, 1], F32)
        nc.tensor.matmul(pcb[:], w16t[:], pcv[:], start=True, stop=True)
        kimg = per.tile([128, 1], F32)
        nc.vector.tensor_scalar(kimg[:], pcb[:], 3.0, None, Op.mult)

        # ---- bisection for per-image rank-k threshold on mrm ----
        lo = per.tile([128, 1], F32)
        nc.vector.memset(lo[:], BISECT_LO)
        mid = per.tile([128, 1], F32)
        cmpd = per.tile([128, TPS], BF16)
        cnt = per.tile([128, 1], F32)
        ge = per.tile([128, 1], mybir.dt.int32)
        # cp = ln(clip(ax / s2)) pieces are interleaved into the bisection
        # loop: DVE runs them in the slack while the PE round-trips.
        r2v = per.tile([128, TPS], F32)
        ratio = per.tile([128, TPS], F32)
        rc = per.tile([128, TPS], F32)

        w = BISECT_W0
        for _t in range(T_BISECT):
            nc.vector.tensor_scalar_add(mid[:], lo[:], w)
            w *= 0.5
            nc.vector.tensor_scalar(
                cmpd[:], mrm[:], mid[:], 0.0, Op.is_ge, Op.add,
                accum_out=cnt[:],
            )
            cps = pp.tile([128, 1], F32, tag="cps")
            nc.tensor.matmul(cps[:], w16t[:], cnt[:], start=True, stop=True)
            if _t == 0:
                nc.vector.reciprocal(r2v[:], s2a[:])
            elif _t == 1:
                nc.vector.tensor_mul(ratio[:], axa[:], r2v[:])
            elif _t == 2:
                nc.vector.tensor_scalar(
                    rc[:], ratio[:], EPS, 1.0 - EPS, Op.max, Op.min
                )
            elif _t == 3:
                nc.scalar.activation(cpt[:], rc[:], Act.Ln)
            nc.vector.tensor_tensor(ge[:], cps[:], kimg[:], op=Op.is_ge)
            nc.vector.copy_predicated(lo[:], ge[:], mid[:])

        # ---- final masked sums ----
        selv = per.tile([128, TPS], BF16)
        nc.vector.tensor_scalar(selv[:], mrm[:], lo[:], None, Op.is_ge)
        sel2 = per.tile([128, TPS], BF16)
        nc.vector.tensor_tensor(sel2[:], selv[:], posm[:], op=Op.add)
        scr = per.tile([128, TPS], F32)
        nc.vector.tensor_mul(scr[:], cpt[:], sel2[:])
        scr2 = per.tile([128, TPS], F32)
        csum = per.tile([128, 1], F32)
        nc.vector.tensor_scalar(
            scr2[:], scr[:], 0.0, 0.0, Op.add, Op.add, accum_out=csum[:]
        )

        # hub partial = 0.5*(sum d2 - sum r1 - sum r2) per partition
        htmp = per.tile([128, NCHUNK], F32)
        nc.vector.tensor_sub(htmp[:], d2s[:], r1s[:])
        nc.vector.tensor_sub(htmp[:], htmp[:], r2s[:])
        hsum = per.tile([128, 1], F32)
        nc.vector.reduce_sum(hsum[:], htmp[:], axis=X)

        pk = per.tile([128, 4], F32)
        nc.vector.memset(pk[:], 0.0)
        nc.vector.tensor_copy(pk[:, 0:1], hsum[:])
        nc.vector.tensor_copy(pk[:, 1:2], csum[:])
        nc.vector.tensor_copy(pk[:, 2:3], pcv[:])
        pkr = pp.tile([128, 4], F32)
        nc.tensor.matmul(pkr[:], onest[:], pk[:], start=True, stop=True)
        outt = per.tile([1, 4], F32)
        i_cp = nc.vector.tensor_copy(outt[:], pkr[0:1, :])
        i_dma = nc.sync.dma_start(out[:], outt[:])

        n1 = nc.sync.nop()
        add_dep_helper(n1.ins, i_cp.ins, sync=True, reason="funnel-dve")
        n2 = nc.sync.nop()
        add_dep_helper(n2.ins, i_dma.ins, sync=True, reason="funnel-dma")

    return nc


def build_bass():
    nc = bass.Bass()
    xl = nc.dram_tensor("xl", [NCHUNK, 128, FC], BF16, kind="ExternalInput")
    al = nc.dram_tensor("al", [NCHUNK, 128, FC], BF16, kind="ExternalInput")
    dl = nc.dram_tensor("dl", [NCHUNK, 128, FD], BF16, kind="ExternalInput")
    w16 = nc.dram_tensor("w16", [128, 128], F32, kind="ExternalInput")
    ones = nc.dram_tensor("ones", [128, 128], F32, kind="ExternalInput")
    out = nc.dram_tensor("out", [1, 4], F32, kind="ExternalOutput")
    emit_program(nc, xl, al, dl, w16, ones, out)
    return _patch_wait_splitting(nc)


def _to_chunks(x, fill):
    """[NIMG, N, D] f32 -> [NCHUNK, 128, TPC*D] bf16 in the
    p=(image,sub) / token-chunk layout."""
    nimg, n, dd = x.shape
    buf = np.full((nimg, NPAD, dd), fill, dtype=np.float32)
    buf[:, :n, :] = x
    # token T = s*TPS + k*TPC + pos
    buf = buf.reshape(nimg, SUBS, NCHUNK, TPC, dd)
    buf = buf.transpose(2, 0, 1, 3, 4)  # (k, i, s, pos, d)
    buf = buf.reshape(NCHUNK, 128, TPC * dd)
    return np.ascontiguousarray(buf.astype(NPBF16))


def kernel(actual_bbox_deltas, actual_labels, pred_bbox_deltas, pred_labels):
    global LAST_RESULTS
    ab = np.asarray(actual_bbox_deltas, dtype=np.float32)
    al_ = np.asarray(actual_labels, dtype=np.float32)
    pb = np.asarray(pred_bbox_deltas, dtype=np.float32)
    pl_ = np.asarray(pred_labels, dtype=np.float32)
    assert pl_.shape == (B, N, C), pl_.shape

    # deltas interleaved per token: (pd0..3, ad0..3)
    pdad = np.concatenate([pb, ab], axis=2)  # [B, N, 8]

    blk = np.arange(128) // SUBS
    w16 = (blk[:, None] == blk[None, :]).astype(np.float32)
    ones = np.ones((128, 128), np.float32)

    nc = build_bass()
    in_maps = []
    for c in range(NCORES):
        sl = slice(c * NIMG, (c + 1) * NIMG)
        in_maps.append(
            {
                "xl": _to_chunks(pl_[sl], PLPAD),
                "al": _to_chunks(al_[sl], 0.0),
                "dl": _to_chunks(pdad[sl], 0.0),
                "w16": w16,
                "ones": ones,
            }
        )

    trace = bool(int(os.environ.get("KERNEL_TRACE", "0")))
    if trace:
        _ensure_ntff_hook()
    res = run_bass_kernel_spmd(
        nc, in_maps, core_ids=list(range(NCORES)), trace=trace
    )
    LAST_RESULTS = res

    hub_sum = 0.0
    cesel_sum = 0.0
    pos_total = 0.0
    for r in res.results:
        o = r["out"].reshape(-1)
        hub_sum += float(o[0])
        cesel_sum += float(o[1])
        pos_total += float(o[2])

    total_pos = max(pos_total, 1.0)
    loc_loss = np.float32(0.25 * 0.5 * hub_sum / total_pos)
    conf_loss = np.float32(-cesel_sum / total_pos)
    return loc_loss, conf_loss


# revision 23
# speedup vs baseline: 2.1395x; 1.0177x over previous
"""SSD-style CustomLoss (Huber loc loss + hard-negative-mined CE conf loss)
as a Trainium2 Bass/Tile kernel, data-parallel over the batch axis on 8
NeuronCores.

v4 design (vs. baseline):
  - bf16 inputs (halves HBM traffic; label one-hots and nonzero-ness of
    deltas are preserved exactly by the cast).
  - CE-from-probs collapses via the one-hot identity to
    cp = ln(clip(<y,x>/sum_c x, eps, 1-eps)) -- no per-class normalize/
    clip/log tensors.
  - Partition layout p = (image, 16 sub-rows); tokens stream in 4 chunks
    of 2208 tokens x 8 images. All heavy elementwise work runs on
    [128, 2898]-shaped tiles (big instructions, few bubbles).
  - Per-token sums over C=21 via bf16 tensor_tensor trees (2x DVE mode)
    instead of 1x tensor_reduce.
  - Huber in exact relu^2 form: hub = 0.5 d^2 - 0.5 relu(d-1)^2
    - 0.5 min(d+1, 0)^2, with Square+accum on the scalar engine.
  - Hard-negative threshold via 12-round bisection; per-image counts with
    ONE tensor_scalar+accum per round thanks to the image-major partition
    layout, cross-partition block-sum via a blockdiag ones matmul.
"""

import os
import sys
import types

import numpy as np

import concourse.bass as bass
import concourse.mybir as mybir
from concourse.bass_utils import run_bass_kernel_spmd
from concourse.mybir import ActivationFunctionType as Act
from concourse.mybir import AluOpType as Op
from concourse.tile import TileContext, add_dep_helper

B, N, C = 64, 8732, 21
NCORES = 8
NIMG = B // NCORES          # images per core
SUBS = 16                   # sub-rows per image -> 8*16 = 128 partitions
TPS = 552                   # tokens per sub-row: 16*552 = 8832 >= 8732
NPAD = SUBS * TPS           # padded tokens per image
NCHUNK = 4
TPC = TPS // NCHUNK         # tokens per partition per chunk = 138
FC = TPC * C                # 2898 label elems per partition per chunk
FD = TPC * 8                # 1104 delta elems per partition per chunk
EPS = 1e-7
BIG = 1.0e30
PLPAD = -20.0               # pl value for padded tokens
T_BISECT = 11
BISECT_LO = 1.0
BISECT_W0 = 8.0             # interval [lo, lo+2w); resolution 16/2^11
F32 = mybir.dt.float32
BF16 = mybir.dt.bfloat16
NPBF16 = np.dtype(mybir.dt.np(mybir.dt.bfloat16))
X = mybir.AxisListType.X

LAST_RESULTS = None

# The walrus build in this container rejects instructions carrying more than
# MAX_WAITS semaphore waits. Tile's scheduler freely emits more, so split the
# excess onto NoOps inserted just before the offending instruction.
MAX_WAITS = 1
NOP_WAITS = 1


def _ensure_ntff_hook():
    """bass_utils' axon trace path imports antenv.axon_hooks, which this
    image lacks. Synthesize the tiny get/set registry and install the
    ctypes-based NTFF hook the boot would have registered."""
    try:
        from antenv.axon_hooks import get_axon_ntff_profile_hook  # noqa: F401

        return
    except ImportError:
        pass
    try:
        import antenv
        from trn_agent_boot.trn_boot import _ntff_profile_via_ctypes

        m = types.ModuleType("antenv.axon_hooks")
        _reg = [None]
        m.set_axon_ntff_profile_hook = lambda h: _reg.__setitem__(0, h)
        m.get_axon_ntff_profile_hook = lambda: _reg[0]
        sys.modules["antenv.axon_hooks"] = m
        antenv.axon_hooks = m
        m.set_axon_ntff_profile_hook(
            _ntff_profile_via_ctypes("/opt/axon/libaxon_pjrt.so")
        )
    except Exception:
        pass


def _split_excess_waits(bir_json: bytes) -> bytes:
    import json as _json

    m = _json.loads(bir_json)
    ctr = 0
    for fdef in m["functions"]:
        for blk in fdef["blocks"]:
            insts = blk["instructions"]
            out = []
            for ins in insts:
                si = ins.get("sync_info")
                ow = (si or {}).get("on_wait") or []
                cap = NOP_WAITS if ins.get("opcode") in ("NoOp", "Drain") else MAX_WAITS
                if len(ow) > cap:
                    keep = ow[-cap:]
                    excess = ow[:-cap]
                    si["on_wait"] = keep
                    while excess:
                        chunk, excess = excess[:NOP_WAITS], excess[NOP_WAITS:]
                        ctr += 1
                        out.append(
                            {
                                "debug": ins.get("debug"),
                                "engine": ins["engine"],
                                "ins": [],
                                "name": f"I-wsplit-{ctr}",
                                "opcode": "NoOp",
                                "outs": [],
                                "sync_info": {"on_update": [], "on_wait": chunk},
                            }
                        )
                out.append(ins)
            blk["instructions"] = out
    return _json.dumps(m).encode()


def _patch_wait_splitting(nc):
    orig = nc.to_json_bytes

    def patched():
        return _split_excess_waits(orig())

    nc.to_json_bytes = patched
    return nc


def _col(ap3, j):
    """[128, T, W] view -> [128, T] view of column j (stride W)."""
    return ap3[:, :, j : j + 1].rearrange("p t o -> p (t o)")


def emit_program(nc, xl, al, dl, w16, ones, out):
    from contextlib import ExitStack

    with TileContext(nc) as tc, ExitStack() as stk:
        per = stk.enter_context(tc.tile_pool(name="per", bufs=1))
        cp_pool = stk.enter_context(tc.tile_pool(name="chunk", bufs=3))
        pp = stk.enter_context(tc.tile_pool(name="ps", bufs=2, space="PSUM"))

        # persistent per-core state
        mrm = per.tile([128, TPS], F32)     # masked ranking values (mr)
        cpt = per.tile([128, TPS], F32)     # ln(clip(x_k / s2)) per token
        posm = per.tile([128, TPS], BF16)   # positive mask
        sall = per.tile([128, TPS, 3], F32)  # (s1, ax, s2) per token
        pc = per.tile([128, NCHUNK], F32)   # per-partition pos counts by chunk
        d2s = per.tile([128, NCHUNK], F32)  # sum d^2*pos by chunk
        r1s = per.tile([128, NCHUNK], F32)  # sum relu(d-1)^2*pos
        r2s = per.tile([128, NCHUNK], F32)  # sum min(d+1,0)^2*pos
        w16t = per.tile([128, 128], F32)
        onest = per.tile([128, 128], F32)

        for k in range(NCHUNK):
            ksl = slice(k * TPC, (k + 1) * TPC)
            xt = cp_pool.tile([128, FC], BF16, tag="xt")
            at = cp_pool.tile([128, FC], BF16, tag="at")
            dt = cp_pool.tile([128, FD], BF16, tag="dt")
            nc.sync.dma_start(xt[:], xl[k])
            nc.sync.dma_start(at[:], al[k])
            nc.sync.dma_start(dt[:], dl[k])

            x3 = xt[:].rearrange("p (t c) -> p t c", c=C)
            a3 = at[:].rearrange("p (t c) -> p t c", c=C)
            d3 = dt[:].rearrange("p (t c) -> p t c", c=8)

            # --- CE stats: s1 = sum exp x, ax = <y,x>, s2 = sum x.
            # Sum over C=21 via a tensor_tensor add tree whose lower levels
            # are merged across the three stats (stat-interleaved scratch). ---
            et = cp_pool.tile([128, FC], BF16, tag="et")
            nc.scalar.activation(et[:], xt[:], Act.Exp)
            axt = cp_pool.tile([128, FC], BF16, tag="axt")
            nc.vector.tensor_mul(axt[:], at[:], xt[:])

            e3 = et[:].rearrange("p (t c) -> p t c", c=C)
            ax3 = axt[:].rearrange("p (t c) -> p t c", c=C)
            t10 = cp_pool.tile([128, TPC, 3, 10], BF16, tag="t10")
            for si, src3 in enumerate((e3, ax3, x3)):
                nc.vector.tensor_tensor(
                    t10[:, :, si, :], src3[:, :, 0:10], src3[:, :, 10:20],
                    op=Op.add,
                )
                # fold the 21st class into slot 0
                c0 = t10[:, :, si, 0:1].rearrange("p t o -> p (t o)")
                nc.vector.tensor_tensor(c0, c0, _col(src3, 20), op=Op.add)
            t5 = cp_pool.tile([128, TPC, 3, 5], BF16, tag="t5")
            nc.vector.tensor_tensor(
                t5[:], t10[:, :, :, 0:5], t10[:, :, :, 5:10], op=Op.add
            )
            t2 = cp_pool.tile([128, TPC, 3, 2], BF16, tag="t2")
            nc.vector.tensor_tensor(
                t2[:], t5[:, :, :, 0:2], t5[:, :, :, 2:4], op=Op.add
            )
            t2a = t2[:, :, :, 0:1].rearrange("p t s o -> p t (s o)")
            t2b = t2[:, :, :, 1:2].rearrange("p t s o -> p t (s o)")
            t5e = t5[:, :, :, 4:5].rearrange("p t s o -> p t (s o)")
            sA = cp_pool.tile([128, TPC, 3], F32, tag="sA")
            nc.vector.tensor_tensor(sA[:], t2a, t2b, op=Op.add)
            nc.vector.tensor_tensor(sall[:, ksl, :], sA[:], t5e, op=Op.add)

            # --- positives: any |actual delta| > 0 (via sum of squares) ---
            sq = cp_pool.tile([128, TPC, 4], BF16, tag="sq")
            nc.scalar.activation(
                sq[:], d3[:, :, 4:8], Act.Square
            )
            p1 = cp_pool.tile([128, TPC, 2], BF16, tag="p1")
            nc.gpsimd.tensor_tensor(p1[:], sq[:, :, 0:2], sq[:, :, 2:4], op=Op.add)
            s4 = cp_pool.tile([128, TPC], F32, tag="s4")
            nc.gpsimd.tensor_tensor(s4[:], _col(p1, 0), _col(p1, 1), op=Op.add)
            nc.vector.tensor_scalar(
                posm[:, ksl], s4[:], 0.0, 0.0, Op.is_gt, Op.add,
                accum_out=pc[:, k : k + 1],
            )

            # --- Huber: hub = 0.5 d^2 - 0.5 relu(d-1)^2 - 0.5 min(d+1,0)^2,
            #     with d pre-masked by pos so negatives contribute 0 ---
            dd = cp_pool.tile([128, TPC, 4], BF16, tag="dd")
            nc.gpsimd.tensor_sub(dd[:], d3[:, :, 0:4], d3[:, :, 4:8])
            dpos = cp_pool.tile([128, TPC, 4], BF16, tag="dpos")
            pos_b = posm[:, ksl][:, :, None].broadcast_to([128, TPC, 4])
            nc.gpsimd.tensor_tensor(dpos[:], dd[:], pos_b, op=Op.mult)

            dsq = cp_pool.tile([128, TPC, 4], BF16, tag="dsq")
            nc.scalar.activation(
                dsq[:], dpos[:], Act.Square, accum_out=d2s[:, k : k + 1]
            )
            re1 = cp_pool.tile([128, TPC, 4], BF16, tag="re1")
            nc.vector.tensor_scalar(re1[:], dpos[:], 1.0, 0.0, Op.subtract, Op.max)
            rsq = cp_pool.tile([128, TPC, 4], BF16, tag="rsq")
            nc.scalar.activation(
                rsq[:], re1[:], Act.Square, accum_out=r1s[:, k : k + 1]
            )
            re2 = cp_pool.tile([128, TPC, 4], BF16, tag="re2")
            nc.vector.tensor_scalar(re2[:], dpos[:], 1.0, 0.0, Op.add, Op.min)
            msq = cp_pool.tile([128, TPC, 4], BF16, tag="msq")
            nc.scalar.activation(
                msq[:], re2[:], Act.Square, accum_out=r2s[:, k : k + 1]
            )

        # ---- weights for cross-partition sums (needed from here on) ----
        nc.sync.dma_start(w16t[:], w16[:])
        nc.sync.dma_start(onest[:], ones[:])

        s1v = sall[:, :, 0:1].rearrange("p t o -> p (t o)")
        axv = sall[:, :, 1:2].rearrange("p t o -> p (t o)")
        s2v = sall[:, :, 2:3].rearrange("p t o -> p (t o)")

        # ---- batched ranking values: mr = ln(s1) - ax, minus BIG at pos ----
        lse = per.tile([128, TPS], F32)
        nc.scalar.activation(lse[:], s1v, Act.Ln)
        mrv = per.tile([128, TPS], F32)
        nc.vector.tensor_sub(mrv[:], lse[:], axv)
        negm = per.tile([128, TPS], BF16)
        nc.vector.tensor_scalar(negm[:], posm[:], -BIG, None, Op.mult)
        nc.vector.tensor_add(mrm[:], mrv[:], negm[:])

        # ---- per-image positive counts, broadcast within image blocks ----
        pcv = per.tile([128, 1], F32)
        nc.vector.reduce_sum(pcv[:], pc[:], axis=X)
        pcb = pp.tile([128, 1], F32)
        nc.tensor.matmul(pcb[:], w16t[:], pcv[:], start=True, stop=True)
        kimg = per.tile([128, 1], F32)
        nc.vector.tensor_scalar(kimg[:], pcb[:], 3.0, None, Op.mult)

        # ---- bisection for per-image rank-k threshold on mrm ----
        lo = per.tile([128, 1], F32)
        nc.vector.memset(lo[:], BISECT_LO)
        mid = per.tile([128, 1], F32)
        cmpd = per.tile([128, TPS], BF16)
        cnt = per.tile([128, 1], F32)
        ge = per.tile([128, 1], mybir.dt.int32)
        # cp = ln(clip(ax/s2, eps, 1-eps)) computed division-free as
        # clip(0.5*(ln ax^2 - ln s2^2), ln eps, ln(1-eps)), patched to
        # ln(eps) where sign(ax) != sign(s2). The DVE pieces are
        # interleaved into the bisection loop (runs in PE round-trip slack);
        # the four Act ops go to the otherwise-idle scalar engine.
        LNEPS = float(np.log(EPS))
        LN1ME = float(np.log1p(-EPS))
        a2t = per.tile([128, TPS], F32)
        b2t = per.tile([128, TPS], F32)
        lat = per.tile([128, TPS], F32)
        lbt = per.tile([128, TPS], F32)
        cp0 = per.tile([128, TPS], F32)
        prod = per.tile([128, TPS], F32)
        sgn = per.tile([128, TPS], mybir.dt.int32)
        lnepst = per.tile([128, TPS], F32)
        nc.scalar.activation(a2t[:], axv, Act.Square)
        nc.scalar.activation(b2t[:], s2v, Act.Square)
        nc.scalar.activation(lat[:], a2t[:], Act.Ln)
        nc.scalar.activation(lbt[:], b2t[:], Act.Ln)

        w = BISECT_W0
        for _t in range(T_BISECT):
            nc.vector.tensor_scalar_add(mid[:], lo[:], w)
            w *= 0.5
            nc.vector.tensor_scalar(
                cmpd[:], mrm[:], mid[:], 0.0, Op.is_ge, Op.add,
                accum_out=cnt[:],
            )
            cps = pp.tile([128, 1], F32, tag="cps")
            nc.tensor.matmul(cps[:], w16t[:], cnt[:], start=True, stop=True)
            if _t == 0:
                nc.vector.memset(lnepst[:], LNEPS)
            elif _t == 1:
                nc.vector.tensor_sub(cp0[:], lat[:], lbt[:])
            elif _t == 2:
                nc.vector.tensor_scalar(
                    cpt[:], cp0[:], 0.5, LNEPS, Op.mult, Op.max
                )
            elif _t == 3:
                nc.vector.tensor_scalar_min(cpt[:], cpt[:], LN1ME)
            elif _t == 4:
                nc.vector.tensor_mul(prod[:], axv, s2v)
            elif _t == 5:
                nc.vector.tensor_scalar(sgn[:], prod[:], 0.0, None, Op.is_lt)
            elif _t == 6:
                nc.vector.copy_predicated(cpt[:], sgn[:], lnepst[:])
            nc.vector.tensor_tensor(ge[:], cps[:], kimg[:], op=Op.is_ge)
            nc.vector.copy_predicated(lo[:], ge[:], mid[:])

        # ---- final masked sums ----
        selv = per.tile([128, TPS], BF16)
        nc.vector.tensor_scalar(selv[:], mrm[:], lo[:], None, Op.is_ge)
        sel2 = per.tile([128, TPS], BF16)
        nc.vector.tensor_tensor(sel2[:], selv[:], posm[:], op=Op.add)
        scr = per.tile([128, TPS], F32)
        nc.vector.tensor_mul(scr[:], cpt[:], sel2[:])
        scr2 = per.tile([128, TPS], F32)
        csum = per.tile([128, 1], F32)
        nc.vector.tensor_scalar(
            scr2[:], scr[:], 0.0, 0.0, Op.add, Op.add, accum_out=csum[:]
        )

        # hub partial = 0.5*(sum d2 - sum r1 - sum r2) per partition
        htmp = per.tile([128, NCHUNK], F32)
        nc.vector.tensor_sub(htmp[:], d2s[:], r1s[:])
        nc.vector.tensor_sub(htmp[:], htmp[:], r2s[:])
        hsum = per.tile([128, 1], F32)
        nc.vector.reduce_sum(hsum[:], htmp[:], axis=X)

        pk = per.tile([128, 4], F32)
        nc.vector.memset(pk[:], 0.0)
        nc.vector.tensor_copy(pk[:, 0:1], hsum[:])
        nc.vector.tensor_copy(pk[:, 1:2], csum[:])
        nc.vector.tensor_copy(pk[:, 2:3], pcv[:])
        pkr = pp.tile([128, 4], F32)
        nc.tensor.matmul(pkr[:], onest[:], pk[:], start=True, stop=True)
        outt = per.tile([1, 4], F32)
        i_cp = nc.vector.tensor_copy(outt[:], pkr[0:1, :])
        i_dma = nc.sync.dma_start(out[:], outt[:])

        n1 = nc.sync.nop()
        add_dep_helper(n1.ins, i_cp.ins, sync=True, reason="funnel-dve")
        n2 = nc.sync.nop()
        add_dep_helper(n2.ins, i_dma.ins, sync=True, reason="funnel-dma")

    return nc


def build_bass():
    nc = bass.Bass()
    xl = nc.dram_tensor("xl", [NCHUNK, 128, FC], BF16, kind="ExternalInput")
    al = nc.dram_tensor("al", [NCHUNK, 128, FC], BF16, kind="ExternalInput")
    dl = nc.dram_tensor("dl", [NCHUNK, 128, FD], BF16, kind="ExternalInput")
    w16 = nc.dram_tensor("w16", [128, 128], F32, kind="ExternalInput")
    ones = nc.dram_tensor("ones", [128, 128], F32, kind="ExternalInput")
    out = nc.dram_tensor("out", [1, 4], F32, kind="ExternalOutput")
    emit_program(nc, xl, al, dl, w16, ones, out)
    return _patch_wait_splitting(nc)


def _to_chunks(x, fill):
    """[NIMG, N, D] f32 -> [NCHUNK, 128, TPC*D] bf16 in the
    p=(image,sub) / token-chunk layout."""
    nimg, n, dd = x.shape
    buf = np.full((nimg, NPAD, dd), fill, dtype=np.float32)
    buf[:, :n, :] = x
    # token T = s*TPS + k*TPC + pos
    buf = buf.reshape(nimg, SUBS, NCHUNK, TPC, dd)
    buf = buf.transpose(2, 0, 1, 3, 4)  # (k, i, s, pos, d)
    buf = buf.reshape(NCHUNK, 128, TPC * dd)
    return np.ascontiguousarray(buf.astype(NPBF16))


def kernel(actual_bbox_deltas, actual_labels, pred_bbox_deltas, pred_labels):
    global LAST_RESULTS
    ab = np.asarray(actual_bbox_deltas, dtype=np.float32)
    al_ = np.asarray(actual_labels, dtype=np.float32)
    pb = np.asarray(pred_bbox_deltas, dtype=np.float32)
    pl_ = np.asarray(pred_labels, dtype=np.float32)
    assert pl_.shape == (B, N, C), pl_.shape

    # deltas interleaved per token: (pd0..3, ad0..3)
    pdad = np.concatenate([pb, ab], axis=2)  # [B, N, 8]

    blk = np.arange(128) // SUBS
    w16 = (blk[:, None] == blk[None, :]).astype(np.float32)
    ones = np.ones((128, 128), np.float32)

    nc = build_bass()
    in_maps = []
    for c in range(NCORES):
        sl = slice(c * NIMG, (c + 1) * NIMG)
        in_maps.append(
            {
                "xl": _to_chunks(pl_[sl], PLPAD),
                "al": _to_chunks(al_[sl], 0.0),
                "dl": _to_chunks(pdad[sl], 0.0),
                "w16": w16,
                "ones": ones,
            }
        )

    trace = bool(int(os.environ.get("KERNEL_TRACE", "0")))
    if trace:
        _ensure_ntff_hook()
    res = run_bass_kernel_spmd(
        nc, in_maps, core_ids=list(range(NCORES)), trace=trace
    )
    LAST_RESULTS = res

    hub_sum = 0.0
    cesel_sum = 0.0
    pos_total = 0.0
    for r in res.results:
        o = r["out"].reshape(-1)
        hub_sum += float(o[0])
        cesel_sum += float(o[1])
        pos_total += float(o[2])

    total_pos = max(pos_total, 1.0)
    loc_loss = np.float32(0.25 * 0.5 * hub_sum / total_pos)
    conf_loss = np.float32(-cesel_sum / total_pos)
    return loc_loss, conf_loss
